# revision 51
# baseline (speedup 1.0000x reference)
"""Trainium2 Bass kernel for batched attention with softmax over the query axis.

Math (per batch element b):
    q = x @ Wq.T + bq ; k = x @ Wk.T + bk ; v = x @ Wv.T + bv
    scores[q,k] = (q . k) / 256
    weights = softmax(scores, axis=q)          # over the QUERY axis
    out[q,h] = sum_k weights[q,k] * v[k,h]

Sharding: pure data parallel — batch B=8 over 8 NeuronCores, one batch
element per core. All feeding/unsharding (including transposes) happens
host-side; the device kernel works on transposed activations:

    xT  [H, S]  (features on partitions)  -> qT, kT [H, S]
    scoresT[k, q] = kT.T @ qT             (softmax axis q == free axis)
    expT = exp(scoresT/256); free-axis row sums give denominators per k
    v[k, :] scaled in place by 1/sum[k]
    outT[h, q] = sum_k v[k, h] * expT[k, q]   -> host transposes back

Engine budget: every bias is a rank-1 accumulating matmul (bias_row.T @ ones
or ones.T @ bias_row), so the scalar engine does nothing but the 32 exp
passes; q/k PSUM->SBUF copies ride the DMA engines. The softmax couples only
over q, which is fully materialized per 128-row k-chunk, so the h-half-0
output accumulation runs inside the scores/exp loop one chunk behind the
exp (flash-style) and the PE never waits on the scalar engine. The h-half-1
output runs as a PE-only pass afterwards to fit PSUM (scores double-buffer +
h0 accumulator = 8 banks). Outputs DMA straight from PSUM.

All matmuls run as float32r (fp32 storage, 1 cycle/row on the PE at
free-dim >= 256); accumulation is fp32 in PSUM.
"""

import numpy as np

import concourse.bass as bass
import concourse.tile as tile
from concourse import bacc, mybir
from concourse.bass_utils import run_bass_kernel_spmd

B, S, H = 8, 2048, 256
P = 128
NH = H // P        # feature chunks (2)
NS = S // P        # sequence chunks (16)
QW = 512           # matmul moving free-dim
NQ = S // QW       # 4
QH = S // 2        # scores-psum half width (1024)
FP = mybir.dt.float32
FPR = mybir.dt.float32r
AF = mybir.ActivationFunctionType


def _r(ap):
    if ap.dtype != mybir.dt.float32r:
        return ap.bitcast(mybir.dt.float32r)
    return ap


def build_nc(niter=1):
    nc = bacc.Bacc("TRN2", target_bir_lowering=False, debug=False)
    xT_d = nc.declare_dram_parameter("xT", [H, S], FPR, isOutput=False)
    wq_d = nc.declare_dram_parameter("WqT", [H, H], FPR, isOutput=False)
    wk_d = nc.declare_dram_parameter("WkT", [H, H], FPR, isOutput=False)
    wv_d = nc.declare_dram_parameter("WvT", [H, H], FPR, isOutput=False)
    # packed [bk | bv | ones] row to load all small constants in one DMA
    cst_d = nc.declare_dram_parameter("consts", [1, 2 * H + QW], FPR,
                                      isOutput=False)
    out_d = nc.declare_dram_parameter("outT", [H, S], FP, isOutput=True)

    with tile.TileContext(nc) as tc:
        # pools are a stack (released LIFO): the ones released mid-iteration
        # must be allocated last (per iteration, below).
        const_pool = tc.alloc_tile_pool(name="const", bufs=1)
        exp_pool = tc.alloc_tile_pool(name="exp", bufs=1)
        stat_pool = tc.alloc_tile_pool(name="stat", bufs=1)
        v_pool = tc.alloc_tile_pool(name="v", bufs=1)
        stage_pool = tc.alloc_tile_pool(name="stage", bufs=2)

        # ---- constants ----
        wq = const_pool.tile([P, NH, H], FPR, tag="wq")
        wk = const_pool.tile([P, NH, H], FPR, tag="wk")
        wv = const_pool.tile([P, NH, H], FPR, tag="wv")
        cst = const_pool.tile([1, 2 * H + QW], FPR, tag="cst")
        bkr = cst[0:1, 0:H]
        bvr = cst[0:1, H:2 * H]
        ones = cst[0:1, 2 * H:2 * H + QW]

        for it in range(niter):
            qk_pool = tc.alloc_tile_pool(name=f"qk{it}", bufs=1)
            x_pool = tc.alloc_tile_pool(name=f"x{it}", bufs=1)
            ps_proj = tc.alloc_tile_pool(name=f"pp{it}", bufs=2, space="PSUM")

            xt = [[x_pool.tile([P, QH], FPR, tag=f"x{h}_{i}",
                               name=f"x{it}_{h}_{i}")
                   for i in range(2)] for h in range(NH)]

            # DMA order = need order (v phase first). One DMA per tensor:
            # the HWDGE pipeline charges ~625 ns fixed per DMA instruction,
            # so fewer, larger transfers shorten the load head.
            nc.sync.dma_start(xt[0][0][:, 0:QW], xT_d[0:P, 0:QW])
            if it == 0:
                nc.scalar.dma_start(
                    wv[:], wv_d.rearrange("(c p) o -> p c o", p=P))
                nc.scalar.dma_start(cst[:], cst_d[:, :])
            nc.sync.dma_start(xt[1][0][:, 0:QW], xT_d[P:2 * P, 0:QW])
            nc.sync.dma_start(xt[0][0][:, QW:QH], xT_d[0:P, QW:QH])
            nc.sync.dma_start(xt[1][0][:, QW:QH], xT_d[P:2 * P, QW:QH])
            if it == 0:
                nc.scalar.dma_start(
                    wq[:], wq_d.rearrange("(c p) o -> p c o", p=P))
            nc.sync.dma_start(xt[0][1][:], xT_d[0:P, QH:S])
            nc.scalar.dma_start(xt[1][1][:], xT_d[P:2 * P, QH:S])
            if it == 0:
                nc.scalar.dma_start(
                    wk[:], wk_d.rearrange("(c p) o -> p c o", p=P))

            q_t = qk_pool.tile([P, NH, S], FPR, tag="qT")
            k_t = qk_pool.tile([P, NH, S], FPR, tag="kT")
            v_t = v_pool.tile([P, NS, H], FPR, tag="v")
            e_t = exp_pool.tile([P, NS, S], FPR, tag="expT")
            sums2 = stat_pool.tile([P, NS, 2], FP, tag="sums2")
            inv = stat_pool.tile([P, NS], FP, tag="inv")

            # v: natural layout [s on partitions, h free]; bias broadcast
            # over partitions via ones.T @ bias_row.
            for sc in range(NS):
                ps = ps_proj.tile([P, H], FP, tag="vps", name=f"pv{it}_{sc}")
                for h in range(NH):
                    lhsT = xt[h][sc // 8][:, (sc % 8) * P:((sc % 8) + 1) * P]
                    nc.tensor.matmul(ps[:], _r(lhsT), wv[:, h, :],
                                     start=(h == 0), stop=False)
                nc.tensor.matmul(ps[:], ones[0:1, 0:P], bvr[:],
                                 start=False, stop=True)
                nc.vector.tensor_copy(v_t[:, sc, :], ps[:])

            # ---- phase 1: projections (PE + DMA only) ----
            # qT/kT: [o on partitions, s free]; bias added as a rank-1
            # accumulating matmul (bias_row.T @ ones_row); PSUM->SBUF copies
            # split across ACT and DVE.
            nd = 0

            def qk_group(wt, br, dst, oc, qh):
                nonlocal nd
                ps = ps_proj.tile([P, QH], FP, tag="qk", bufs=3,
                                  name=f"pj{it}_{oc}_{qh}_{id(wt) % 97}")
                for h in range(NH):
                    lhsT = wt[:, h, oc * P:(oc + 1) * P]
                    for j in range(2):
                        nc.tensor.matmul(
                            ps[:, j * QW:(j + 1) * QW],
                            _r(lhsT),
                            xt[h][qh][:, j * QW:(j + 1) * QW],
                            start=(h == 0),
                            stop=(br is None and h == NH - 1),
                        )
                if br is not None:
                    for j in range(2):
                        nc.tensor.matmul(
                            ps[:, j * QW:(j + 1) * QW],
                            br[0:1, oc * P:(oc + 1) * P],
                            ones[:],
                            start=False,
                            stop=True,
                        )
                cdst = dst[:, oc, qh * QH:(qh + 1) * QH]
                if nd % 2 == 0:
                    nc.scalar.copy(cdst, ps[:])
                else:
                    nc.vector.tensor_copy(cdst, ps[:])
                nd += 1

            def scores_half(kc, qh, pool):
                ps = pool.tile([P, QH], FP, tag=pool is ps_proj and "qk" or "sc",
                               bufs=3 if pool is ps_proj else None,
                               name=f"sc{it}_{kc}_{qh}")
                for h in range(NH):
                    lhsT = k_t[:, h, kc * P:(kc + 1) * P]
                    for j in range(2):
                        q0 = qh * QH + j * QW
                        nc.tensor.matmul(
                            ps[:, j * QW:(j + 1) * QW],
                            _r(lhsT),
                            _r(q_t[:, h, q0:q0 + QW]),
                            start=(h == 0),
                            stop=(h == NH - 1),
                        )
                nc.scalar.activation(
                    e_t[:, kc, qh * QH:(qh + 1) * QH], ps[:], AF.Exp,
                    bias=0.0, scale=1.0 / float(H),
                    accum_out=sums2[:, kc, qh:qh + 1])

            # qh=0 groups first: the pre-warm scores half only needs these
            for oc in range(NH):
                qk_group(wq, None, q_t, oc, 0)
            for oc in range(NH):
                qk_group(wk, bkr, k_t, oc, 0)
            # pre-warm: first scores half in a projection-pool slot; its exp
            # runs while the PE does the qh=1 projection groups below
            scores_half(0, 0, ps_proj)
            for oc in range(NH):
                qk_group(wq, None, q_t, oc, 1)
            for oc in range(NH):
                qk_group(wk, bkr, k_t, oc, 1)

            x_pool.release()
            ps_proj.release()

            # ---- fused phase: scoresT -> exp -> h-half-0 output accum ----
            # PSUM: out0 accumulator (4 banks) + scores halves (2 x 2 banks).
            # Output matmuls trail the exp by one k-chunk so the PE never
            # waits on the exp -> rowsum -> reciprocal -> v-scale chain.
            ps_out0 = tc.alloc_tile_pool(name=f"po{it}", bufs=1, space="PSUM")
            ps_sc = tc.alloc_tile_pool(name=f"sc{it}", bufs=2, space="PSUM")
            out0 = ps_out0.tile([P, S], FP, tag="o0", name=f"o0_{it}")

            def out0_mms(kc):
                for i in range(NQ):
                    nc.tensor.matmul(
                        out0[:, i * QW:(i + 1) * QW],
                        _r(v_t[:, kc, 0:P]),
                        _r(e_t[:, kc, i * QW:(i + 1) * QW]),
                        start=(kc == 0),
                        stop=(kc == NS - 1),
                    )

            for kc in range(NS):
                for qh in range(2):
                    if kc == 0 and qh == 0:
                        continue  # pre-warmed in the projection phase
                    scores_half(kc, qh, ps_sc)
                nc.vector.tensor_add(
                    inv[:, kc:kc + 1], sums2[:, kc, 0:1], sums2[:, kc, 1:2])
                nc.vector.reciprocal(inv[:, kc:kc + 1], inv[:, kc:kc + 1])
                # fold softmax denominator into v rows (64x cheaper than
                # scaling the [S, S] weight matrix)
                nc.vector.tensor_scalar_mul(
                    v_t[:, kc, :], v_t[:, kc, :], inv[:, kc:kc + 1])
                if kc >= 2:
                    out0_mms(kc - 2)
            out0_mms(NS - 2)
            out0_mms(NS - 1)

            qk_pool.release()
            ps_sc.release()

            # flush h-half 0 (overlaps the h-half-1 pass below)
            for i in range(NQ):
                st = stage_pool.tile([P, QW], FP, tag="stage",
                                     name=f"s0_{it}_{i}")
                nc.scalar.copy(st[:], out0[:, i * QW:(i + 1) * QW])
                nc.sync.dma_start(out_d[0:P, i * QW:(i + 1) * QW], st[:])

            # ---- h-half-1 output: pure PE pass, per-q-slice accumulate ----
            ps_out1 = tc.alloc_tile_pool(name=f"p1{it}", bufs=2, space="PSUM")
            for i in range(NQ):
                ps = ps_out1.tile([P, QW], FP, tag="o1", bufs=3,
                                   name=f"o1_{it}_{i}")
                for kc in range(NS):
                    nc.tensor.matmul(
                        ps[:],
                        _r(v_t[:, kc, P:2 * P]),
                        _r(e_t[:, kc, i * QW:(i + 1) * QW]),
                        start=(kc == 0),
                        stop=(kc == NS - 1),
                    )
                st = stage_pool.tile([P, QW], FP, tag="stage",
                                     name=f"s1_{it}_{i}")
                nc.vector.tensor_copy(st[:], ps[:])
                nc.sync.dma_start(out_d[P:2 * P, i * QW:(i + 1) * QW], st[:])

            ps_out1.release()
            ps_out0.release()

        stage_pool.release()
        v_pool.release()
        stat_pool.release()
        exp_pool.release()
        const_pool.release()

    nc.finalize()
    return nc


_NC_CACHE = None


def _get_nc():
    global _NC_CACHE
    if _NC_CACHE is None:
        _NC_CACHE = build_nc()
    return _NC_CACHE


def _run(in_maps, trace=False, **kw):
    nc = _get_nc()
    return run_bass_kernel_spmd(nc, in_maps, core_ids=list(range(B)),
                                trace=trace, **kw)


def make_in_maps(inputs, Wq, bq, Wk, bk, Wv, bv):
    f32 = lambda a: np.ascontiguousarray(np.asarray(a), dtype=np.float32)
    WqT = f32(np.asarray(Wq).T)
    WkT = f32(np.asarray(Wk).T)
    WvT = f32(np.asarray(Wv).T)
    consts = np.concatenate(
        [f32(np.asarray(bk).reshape(1, H)),
         f32(np.asarray(bv).reshape(1, H)),
         np.ones((1, QW), dtype=np.float32)], axis=1)
    return [
        {"xT": f32(np.asarray(inputs[b]).T), "WqT": WqT, "WkT": WkT,
         "WvT": WvT, "consts": consts}
        for b in range(B)
    ]


def kernel(inputs, Wq, bq, Wk, bk, Wv, bv):
    in_maps = make_in_maps(inputs, Wq, bq, Wk, bk, Wv, bv)
    res = _run(in_maps, trace=False)
    out = np.stack([np.asarray(res.results[b]["outT"]).T for b in range(B)])
    return np.ascontiguousarray(out.astype(np.float32))
